# revision 9
# baseline (speedup 1.0000x reference)
"""NT-Xent (SimCLR) contrastive loss on 8 Trainium2 NeuronCores.

Math: with z = concat(z_i, z_j) [2B, D], zn = z / ||z||_row,
logits = zn @ zn.T / T (diag masked), targets pair row r with r+-B.

loss = mean_r( LSE_r - l_r )
     = mean_r( log(S~_r) + C - 2*p_r )
where S~_r = sum_{c != r} exp(2*s_rc - C),  computed as
      S_r (full row sum incl. diag) - exp(2*d_r - C),
  s_rc = zn_r . zn_c,  d_r = zn_r . zn_r (self dot, ~1),
  p_r = zn_r . zn_partner(r),  C = 2.0 = 1/T (max possible logit).

Sharding: data-parallel over rows of the similarity matrix.  Every core
receives the FULL z, pre-rotated by 1024*k rows (host-side np.roll) so the
SPMD program always works on "rows 0..1023" — no per-core addressing.
Rotation is a permutation, so row sums over all 8192 columns are invariant,
and partner(r) = (r + 4096) % 8192 is rotation-invariant.

Device pipeline per core:
  1. gpsimd cast-DMA: z fp32 HBM -> bf16 SBUF natural chunks [128, 2048]
  2. per chunk: ss via tensor_tensor_reduce;  inv = Exp(-0.5 * Ln(ss)) (ACT,
     single exp/ln table set);  zn = z * inv (DVE broadcast);  16 DMA xbar
     transposes -> znT tiles [128, 2048] (columns = rotated rows)
  3. main loop over 4 column groups x 8 own row tiles: 8 bf16 matmuls
     (K = 2 x 128) -> PSUM [128, 2048];  ACT Exp(scale=2, bias=-2) with
     accum_out giving per-row partial sums
  4. pair/self dots via tensor_tensor_reduce on natural chunks 0 and 4
Host combines 8 cores' partial vectors: log, mean  (the all-reduce of the
sharding hint, done at unshard time).
"""

import numpy as np

import concourse.bacc as bacc
import concourse.mybir as mybir
import concourse.tile as tile
from concourse.bass_utils import run_bass_kernel_spmd

P = 128
D = 256
B = 4096
N2 = 2 * B            # 8192 rows total
NCORES = 8
NCH = 8               # row chunks of 1024
TPC = 8               # [128, D] tiles per chunk
CHW = TPC * D         # 2048: chunk width in natural layout
G = 4                 # column groups of 2048 in znT
M_TILES = 8           # own 1024 rows = 8 M-subtiles
TEMP = 0.5
CSTAB = 2.0           # stabilization constant = 1/T

f32 = mybir.dt.float32
bf16 = mybir.dt.bfloat16
AF = mybir.ActivationFunctionType
OP = mybir.AluOpType

import os
# Stage gate for HW bisection: 1=casts, 2=+ss/inv, 3=+normalize,
# 4=+transposes+d/p, 5=+main-loop-noaccum, 6=full (default)
STAGE = int(os.environ.get("KERNEL_STAGE", "6"))


def _emit(tc, z, s_out, d_out, p_out):
    nc = tc.nc

    with tc.tile_pool(name="raw", bufs=3) as raw_pool, \
            tc.tile_pool(name="zn", bufs=NCH) as zn_pool, \
            tc.tile_pool(name="znt", bufs=2 * G) as znt_pool, \
            tc.tile_pool(name="small", bufs=3) as small_pool, \
            tc.tile_pool(name="ttrash", bufs=3) as ttrash_pool, \
            tc.tile_pool(name="etrash", bufs=2) as etrash_pool, \
            tc.tile_pool(name="acc", bufs=1) as acc_pool, \
            tc.tile_pool(name="psum", bufs=2, space="PSUM") as psum_pool:
        # znT[h][g]: columns 2048*g .. of the transposed normalized z for
        # d-half h.  8 tiles, all live for the whole kernel.
        znt = [[znt_pool.tile([P, CHW], bf16, tag="znt", name=f"znt{h}_{g}") for g in range(G)]
               for h in range(2)]

        rs_buf = acc_pool.tile([P, M_TILES * G], f32, tag="rs", name="rs_buf")
        bias_t = acc_pool.tile([P, 1], f32, tag="bias", name="bias_t")
        nc.vector.memset(bias_t[:], -CSTAB)
        s_sb = acc_pool.tile([P, M_TILES], f32, tag="ssb", name="s_sb")
        d_sb = acc_pool.tile([P, TPC], f32, tag="dsb", name="d_sb")
        p_sb = acc_pool.tile([P, TPC], f32, tag="psb", name="p_sb")

        for t in (s_sb, d_sb, p_sb):
            nc.vector.memset(t[:], 1.0)
        nc.vector.memset(rs_buf[:], 1.0)

        zns = []
        for c in range(NCH):
            raw = raw_pool.tile([P, CHW], bf16, tag="raw", name=f"raw{c}")
            src = z[1024 * c:1024 * (c + 1), :].rearrange(
                "(n p) d -> p n d", p=P)
            nc.gpsimd.dma_start(
                out=raw[:].rearrange("p (n d) -> p n d", d=D), in_=src)

            if STAGE < 2:
                continue
            ss8 = small_pool.tile([P, TPC], f32, tag="ss8", name=f"ss8_{c}")
            for t in range(TPC):
                tt = ttrash_pool.tile([P, D], bf16, tag="tt", name="tt")
                nc.vector.tensor_tensor(
                    tt[:], raw[:, D * t:D * (t + 1)],
                    raw[:, D * t:D * (t + 1)], op=OP.mult)
                nc.vector.reduce_sum(
                    out=ss8[:, t:t + 1], in_=tt[:],
                    axis=mybir.AxisListType.X)
            # inv = ss^(-1/2) = Exp(-0.5 * Ln(ss)); stays in the exp/ln
            # ACT table set shared with the main-loop Exp.
            ln8 = small_pool.tile([P, TPC], f32, tag="ln8", name=f"ln8_{c}")
            nc.scalar.activation(ln8[:], ss8[:], AF.Ln)
            inv8 = small_pool.tile([P, TPC], bf16, tag="inv8", name=f"inv8_{c}")
            nc.scalar.activation(inv8[:], ln8[:], AF.Exp, scale=-0.5)

            if STAGE < 3:
                continue
            zn = zn_pool.tile([P, CHW], bf16, tag="zn", name=f"zn{c}")
            nc.vector.tensor_tensor(
                out=zn[:].rearrange("p (t d) -> p t d", t=TPC),
                in0=raw[:].rearrange("p (t d) -> p t d", t=TPC),
                in1=inv8[:].unsqueeze(-1).broadcast_to([P, TPC, D]),
                op=OP.mult)
            zns.append(zn)

            if STAGE < 4:
                continue
            g = c // 2
            off = 1024 * (c % 2)
            for t in range(TPC):
                for h in range(2):
                    nc.sync.dma_start(
                        out=znt[h][g][:, off + 128 * t: off + 128 * (t + 1)],
                        in_=zn[:, D * t + 128 * h: D * t + 128 * (h + 1)],
                        transpose=True)

        # Self dots (match the matmul's bf16 diagonal) and pair dots.
        for t in range(TPC if STAGE >= 4 else 0):
            tt = ttrash_pool.tile([P, D], bf16, tag="tt", name="tt")
            nc.vector.tensor_tensor(
                tt[:], zns[0][:, D * t:D * (t + 1)],
                zns[0][:, D * t:D * (t + 1)], op=OP.mult)
            nc.vector.reduce_sum(
                out=d_sb[:, t:t + 1], in_=tt[:], axis=mybir.AxisListType.X)
            tt2 = ttrash_pool.tile([P, D], bf16, tag="tt", name="tt")
            nc.vector.tensor_tensor(
                tt2[:], zns[0][:, D * t:D * (t + 1)],
                zns[4][:, D * t:D * (t + 1)], op=OP.mult)
            nc.vector.reduce_sum(
                out=p_sb[:, t:t + 1], in_=tt2[:], axis=mybir.AxisListType.X)

        # Main loop: rows 0..1023 (own) x all 8192 columns.
        if STAGE >= 5:
            for g in range(G):
                for m in range(M_TILES):
                    ps = psum_pool.tile([P, CHW], f32, tag="ps",
                                        name=f"ps{g}_{m}")
                    for h in range(2):
                        for c4 in range(4):
                            nc.tensor.matmul(
                                out=ps[:, 512 * c4:512 * (c4 + 1)],
                                lhsT=znt[h][0][:, 128 * m:128 * (m + 1)],
                                rhs=znt[h][g][:, 512 * c4:512 * (c4 + 1)],
                                start=(h == 0), stop=(h == 1))
                    et = etrash_pool.tile([P, CHW], bf16, tag="et",
                                          name=f"et{g}_{m}")
                    idx = 4 * m + g
                    if STAGE == 5:
                        nc.scalar.activation(
                            et[:], ps[:], AF.Exp, bias=bias_t[:],
                            scale=1.0 / TEMP)
                    else:
                        nc.scalar.activation(
                            et[:], ps[:], AF.Exp, bias=bias_t[:],
                            scale=1.0 / TEMP,
                            accum_out=rs_buf[:, idx:idx + 1])

        nc.vector.reduce_sum(
            out=s_sb[:].unsqueeze(-1),
            in_=rs_buf[:].rearrange("p (m g) -> p m g", g=G),
            axis=mybir.AxisListType.X)

        nc.sync.dma_start(out=s_out, in_=s_sb[:])
        nc.sync.dma_start(out=d_out, in_=d_sb[:])
        nc.sync.dma_start(out=p_out, in_=p_sb[:])


def build():
    nc = bacc.Bacc("TRN2", target_bir_lowering=False, debug=False)
    z = nc.dram_tensor("z", [N2, D], f32, kind="ExternalInput").ap()
    s_out = nc.dram_tensor("s_out", [P, M_TILES], f32, kind="ExternalOutput").ap()
    d_out = nc.dram_tensor("d_out", [P, TPC], f32, kind="ExternalOutput").ap()
    p_out = nc.dram_tensor("p_out", [P, TPC], f32, kind="ExternalOutput").ap()
    with tile.TileContext(nc) as tc:
        _emit(tc, z, s_out, d_out, p_out)
    nc.compile()
    return nc


def make_in_maps(z_i, z_j):
    z_full = np.concatenate(
        [np.asarray(z_i, dtype=np.float32), np.asarray(z_j, dtype=np.float32)],
        axis=0)
    return [{"z": np.ascontiguousarray(np.roll(z_full, -1024 * k, axis=0))}
            for k in range(NCORES)]


def combine(results):
    S = np.empty(N2, np.float64)
    dv = np.empty(N2, np.float64)
    pv = np.empty(N2, np.float64)
    pp = np.arange(P)[:, None]
    mm = np.arange(M_TILES)[None, :]
    for k in range(NCORES):
        gidx = ((1024 * k + 128 * mm + pp) % N2).ravel()
        S[gidx] = results[k]["s_out"].astype(np.float64).ravel()
        dv[gidx] = results[k]["d_out"].astype(np.float64).ravel()
        pv[gidx] = results[k]["p_out"].astype(np.float64).ravel()
    St = S - np.exp(dv / TEMP - CSTAB)
    lse = np.log(St) + CSTAB
    loss = np.mean(lse - pv / TEMP)
    return np.asarray(loss, dtype=np.float32)


_NC_CACHE = None


def kernel(z_i, z_j):
    global _NC_CACHE
    if _NC_CACHE is None:
        _NC_CACHE = build()
    res = run_bass_kernel_spmd(
        _NC_CACHE, make_in_maps(z_i, z_j), list(range(NCORES))).results
    return combine(res)


# revision 10
# speedup vs baseline: 1.4000x; 1.4000x over previous
"""NT-Xent (SimCLR) contrastive loss on 8 Trainium2 NeuronCores.

Math: with z = concat(z_i, z_j) [2B, D], zn = z / ||z||_row,
logits = zn @ zn.T / T (diag masked), targets pair row r with r+-B.

loss = mean_r( LSE_r - l_r )
     = mean_r( log(S~_r) + C - 2*p_r )
where S~_r = sum_{c != r} exp(2*s_rc - C),  computed as
      S_r (full row sum incl. diag) - exp(2*d_r - C),
  s_rc = zn_r . zn_c,  d_r = zn_r . zn_r (self dot, ~1),
  p_r = zn_r . zn_partner(r),  C = 2.0 = 1/T (max possible logit).

Sharding: data-parallel over rows of the similarity matrix.  Every core
receives the FULL z, pre-rotated by 1024*k rows (host-side np.roll) so the
SPMD program always works on "rows 0..1023" — no per-core addressing.
Rotation is a permutation, so row sums over all 8192 columns are invariant,
and partner(r) = (r + 4096) % 8192 is rotation-invariant.

Device pipeline per core:
  1. gpsimd cast-DMA: z fp32 HBM -> bf16 SBUF natural chunks [128, 2048]
  2. per chunk: ss via tensor_tensor_reduce;  inv = Exp(-0.5 * Ln(ss)) (ACT,
     single exp/ln table set);  zn = z * inv (DVE broadcast);  16 DMA xbar
     transposes -> znT tiles [128, 2048] (columns = rotated rows)
  3. main loop over 4 column groups x 8 own row tiles: 8 bf16 matmuls
     (K = 2 x 128) -> PSUM [128, 2048];  ACT Exp(scale=2, bias=-2) with
     accum_out giving per-row partial sums
  4. pair/self dots via tensor_tensor_reduce on natural chunks 0 and 4
Host combines 8 cores' partial vectors: log, mean  (the all-reduce of the
sharding hint, done at unshard time).
"""

import numpy as np

import concourse.bacc as bacc
import concourse.mybir as mybir
import concourse.tile as tile
from concourse.bass_utils import run_bass_kernel_spmd

P = 128
D = 256
B = 4096
N2 = 2 * B            # 8192 rows total
NCORES = 8
NCH = 8               # row chunks of 1024
TPC = 8               # [128, D] tiles per chunk
CHW = TPC * D         # 2048: chunk width in natural layout
G = 4                 # column groups of 2048 in znT
M_TILES = 8           # own 1024 rows = 8 M-subtiles
TEMP = 0.5
CSTAB = 2.0           # stabilization constant = 1/T

f32 = mybir.dt.float32
bf16 = mybir.dt.bfloat16
AF = mybir.ActivationFunctionType
OP = mybir.AluOpType

import os
# Stage gate for HW bisection: 1=casts, 2=+ss/inv, 3=+normalize,
# 4=+transposes+d/p, 5=+main-loop-noaccum, 6=full (default)
STAGE = int(os.environ.get("KERNEL_STAGE", "6"))


def _emit(tc, z, s_out, d_out, p_out):
    nc = tc.nc

    with tc.tile_pool(name="raw", bufs=NCH) as raw_pool, \
            tc.tile_pool(name="zn", bufs=NCH) as zn_pool, \
            tc.tile_pool(name="znt", bufs=2 * G) as znt_pool, \
            tc.tile_pool(name="small", bufs=3) as small_pool, \
            tc.tile_pool(name="ttrash", bufs=3) as ttrash_pool, \
            tc.tile_pool(name="etrash", bufs=2) as etrash_pool, \
            tc.tile_pool(name="acc", bufs=1) as acc_pool, \
            tc.tile_pool(name="dram", bufs=NCH, space="DRAM") as dram_pool, \
            tc.tile_pool(name="psum", bufs=2, space="PSUM") as psum_pool:
        # znT[h][g]: columns 2048*g .. of the transposed normalized z for
        # d-half h.  8 tiles, all live for the whole kernel.
        znt = [[znt_pool.tile([P, CHW], bf16, tag="znt", name=f"znt{h}_{g}") for g in range(G)]
               for h in range(2)]

        rs_buf = acc_pool.tile([P, M_TILES * G], f32, tag="rs", name="rs_buf")
        bias_t = acc_pool.tile([P, 1], f32, tag="bias", name="bias_t")
        nc.vector.memset(bias_t[:], -CSTAB)
        ss_all = acc_pool.tile([P, NCH * TPC], f32, tag="ssall", name="ss_all")
        inv_all = acc_pool.tile([P, NCH * TPC], bf16, tag="invall", name="inv_all")
        s_sb = acc_pool.tile([P, M_TILES], f32, tag="ssb", name="s_sb")
        d_sb = acc_pool.tile([P, TPC], f32, tag="dsb", name="d_sb")
        p_sb = acc_pool.tile([P, TPC], f32, tag="psb", name="p_sb")

        for t in (s_sb, d_sb, p_sb):
            nc.vector.memset(t[:], 1.0)
        nc.vector.memset(rs_buf[:], 1.0)

        raws = []
        for c in range(NCH):
            raw = raw_pool.tile([P, CHW], bf16, tag="raw", name=f"raw{c}")
            src = z[1024 * c:1024 * (c + 1), :].rearrange(
                "(n p) d -> p n d", p=P)
            nc.gpsimd.dma_start(
                out=raw[:].rearrange("p (n d) -> p n d", d=D), in_=src)
            raws.append(raw)
            if STAGE < 2:
                continue
            sq = ttrash_pool.tile([P, CHW], bf16, tag="tt", name="tt")
            nc.vector.tensor_tensor(sq[:], raw[:], raw[:], op=OP.mult)
            nc.vector.reduce_sum(
                out=ss_all[:, TPC * c:TPC * (c + 1)].unsqueeze(-1),
                in_=sq[:].rearrange("p (t d) -> p t d", t=TPC),
                axis=mybir.AxisListType.X)

        if STAGE >= 2:
            # One Ln + one Exp for all 64 norms: inv = Exp(-0.5*Ln(ss)).
            ln_all = small_pool.tile([P, NCH * TPC], f32, tag="lnall",
                                     name="ln_all")
            nc.scalar.activation(ln_all[:], ss_all[:], AF.Ln)
            nc.scalar.activation(inv_all[:], ln_all[:], AF.Exp, scale=-0.5)

        zns = []
        for c in range(NCH):
            if STAGE < 3:
                continue
            zn = zn_pool.tile([P, CHW], bf16, tag="zn", name=f"zn{c}")
            nc.vector.tensor_tensor(
                out=zn[:].rearrange("p (t d) -> p t d", t=TPC),
                in0=raws[c][:].rearrange("p (t d) -> p t d", t=TPC),
                in1=inv_all[:, TPC * c:TPC * (c + 1)].unsqueeze(-1)
                    .broadcast_to([P, TPC, D]),
                op=OP.mult)
            zns.append(zn)

            if STAGE < 4:
                continue
            # Bounce zn to DRAM, then two big DRAM-source xbar transposes.
            zb = dram_pool.tile([1024, D], bf16, tag="zb", name=f"zb{c}")
            nc.sync.dma_start(
                out=zb[:].rearrange("(t p) d -> p t d", p=P),
                in_=zn[:].rearrange("p (t d) -> p t d", t=TPC))
            g = c // 2
            off = 1024 * (c % 2)
            for h in range(2):
                nc.scalar.dma_start(
                    out=znt[h][g][:, off:off + 1024],
                    in_=zb[:, 128 * h:128 * (h + 1)],
                    transpose=True)

        # Self dots (match the matmul's bf16 diagonal) and pair dots.
        if STAGE >= 4:
            for src0, src1, dst in ((zns[0], zns[0], d_sb),
                                    (zns[0], zns[4], p_sb)):
                tt = ttrash_pool.tile([P, CHW], bf16, tag="tt", name="tt")
                nc.vector.tensor_tensor(tt[:], src0[:], src1[:], op=OP.mult)
                nc.vector.reduce_sum(
                    out=dst[:].unsqueeze(-1),
                    in_=tt[:].rearrange("p (t d) -> p t d", t=TPC),
                    axis=mybir.AxisListType.X)

        # Main loop: rows 0..1023 (own) x all 8192 columns.
        if STAGE >= 5:
            for g in range(G):
                for m in range(M_TILES):
                    ps = psum_pool.tile([P, CHW], f32, tag="ps",
                                        name=f"ps{g}_{m}")
                    for h in range(2):
                        for c4 in range(4):
                            nc.tensor.matmul(
                                out=ps[:, 512 * c4:512 * (c4 + 1)],
                                lhsT=znt[h][0][:, 128 * m:128 * (m + 1)],
                                rhs=znt[h][g][:, 512 * c4:512 * (c4 + 1)],
                                start=(h == 0), stop=(h == 1))
                    et = etrash_pool.tile([P, CHW], bf16, tag="et",
                                          name=f"et{g}_{m}")
                    idx = 4 * m + g
                    if STAGE == 5:
                        nc.scalar.activation(
                            et[:], ps[:], AF.Exp, bias=bias_t[:],
                            scale=1.0 / TEMP)
                    else:
                        nc.scalar.activation(
                            et[:], ps[:], AF.Exp, bias=bias_t[:],
                            scale=1.0 / TEMP,
                            accum_out=rs_buf[:, idx:idx + 1])

        nc.vector.reduce_sum(
            out=s_sb[:].unsqueeze(-1),
            in_=rs_buf[:].rearrange("p (m g) -> p m g", g=G),
            axis=mybir.AxisListType.X)

        nc.sync.dma_start(out=s_out, in_=s_sb[:])
        nc.sync.dma_start(out=d_out, in_=d_sb[:])
        nc.sync.dma_start(out=p_out, in_=p_sb[:])


def build():
    nc = bacc.Bacc("TRN2", target_bir_lowering=False, debug=False)
    z = nc.dram_tensor("z", [N2, D], f32, kind="ExternalInput").ap()
    s_out = nc.dram_tensor("s_out", [P, M_TILES], f32, kind="ExternalOutput").ap()
    d_out = nc.dram_tensor("d_out", [P, TPC], f32, kind="ExternalOutput").ap()
    p_out = nc.dram_tensor("p_out", [P, TPC], f32, kind="ExternalOutput").ap()
    with tile.TileContext(nc) as tc:
        _emit(tc, z, s_out, d_out, p_out)
    nc.compile()
    return nc


def make_in_maps(z_i, z_j):
    z_full = np.concatenate(
        [np.asarray(z_i, dtype=np.float32), np.asarray(z_j, dtype=np.float32)],
        axis=0)
    return [{"z": np.ascontiguousarray(np.roll(z_full, -1024 * k, axis=0))}
            for k in range(NCORES)]


def combine(results):
    S = np.empty(N2, np.float64)
    dv = np.empty(N2, np.float64)
    pv = np.empty(N2, np.float64)
    pp = np.arange(P)[:, None]
    mm = np.arange(M_TILES)[None, :]
    for k in range(NCORES):
        gidx = ((1024 * k + 128 * mm + pp) % N2).ravel()
        S[gidx] = results[k]["s_out"].astype(np.float64).ravel()
        dv[gidx] = results[k]["d_out"].astype(np.float64).ravel()
        pv[gidx] = results[k]["p_out"].astype(np.float64).ravel()
    St = S - np.exp(dv / TEMP - CSTAB)
    lse = np.log(St) + CSTAB
    loss = np.mean(lse - pv / TEMP)
    return np.asarray(loss, dtype=np.float32)


_NC_CACHE = None


def kernel(z_i, z_j):
    global _NC_CACHE
    if _NC_CACHE is None:
        _NC_CACHE = build()
    res = run_bass_kernel_spmd(
        _NC_CACHE, make_in_maps(z_i, z_j), list(range(NCORES))).results
    return combine(res)
